# revision 1
# baseline (speedup 1.0000x reference)
"""Trainium2 Bass kernel for nn_DecoderModel_12352325943321.

6-layer post-LN decoder, SwiGLU FFN, per-head staggered windowed causal
attention (head h window (h+1)*64), tied-embedding lm_head.
B=2, S=1024, D=512, H=8, L=6, V=32000.

Sharding (8 NeuronCores): DP-2 over batch (cores 0-3 batch 0, cores 4-7
batch 1) x TP-4 within each group:
  - attention heads: rank j owns heads (j, 7-j); AllGather head outputs,
    Wo computed replicated on each rank.
  - FFN: rank j owns 512 of 2048 hidden units (matching u/g slices);
    AllReduce of W2 partial sums.
  - lm_head: rank j owns vocab rows [j*8000, (j+1)*8000); host concat.

On-core layout: activations feature-major [D(part), S(free)]; fp32 residual
stream + bf16 matmul copies; weights host-pretransposed [K, M] bf16; fp32
PSUM accumulation.  The SPMD graph is rank-independent: all per-rank
differences (head windows, weight slices, masks) live in input data; the
attention loop runs a fixed 22 k-tile schedule with per-rank additive masks
(-1e30 pads) so every core executes the same NEFF.
"""
import os
from contextlib import ExitStack
import numpy as np
import ml_dtypes

import concourse.bass as bass
import concourse.bacc as bacc
import concourse.tile as tile
import concourse.mybir as mybir
from concourse import bass_utils
from concourse.masks import make_identity

f32 = mybir.dt.float32
bf16 = mybir.dt.bfloat16
i32 = mybir.dt.int32
AF = mybir.ActivationFunctionType
ALU = mybir.AluOpType

B, S, D, H, L, V, W, FF = 2, 1024, 512, 8, 6, 32000, 64, 2048
HD = D // H          # 64
TP = 4               # tensor-parallel ranks per group
NC = 8
VS = V // TP         # 8000 vocab rows per rank
NT = S // 128        # 8 token tiles
ND = D // 128        # 4 feature tiles
NSP = S // 512       # 2 token spans of 512
NVS = VS // 500      # 16 vocab subtiles
NEG = -1e30
EPS = 1e-5

# fixed attention (span, ktile) schedule per head slot; A: win<=256, B: <=512
SLOT_KT = {
    0: [(0, t) for t in range(0, 4)] + [(1, t) for t in range(2, 8)],
    1: [(0, t) for t in range(0, 4)] + [(1, t) for t in range(0, 8)],
}
NMASK = len(SLOT_KT[0]) + len(SLOT_KT[1])  # 22

_CACHE = {}
LAST_RESULTS = None


# ----------------------------------------------------------------- build
def build_nc(n_layers=L, taps=()):
    nc = bacc.Bacc("TRN2", target_bir_lowering=False, debug=False,
                   enable_asserts=True, num_devices=NC)

    def din(name, shape, dt):
        return nc.dram_tensor(name, shape, dt, kind="ExternalInput").ap()

    E = {
        "n_layers": n_layers,
        "groups": [[0, 1, 2, 3], [4, 5, 6, 7]],
        "ids_ap": din("ids", [128, NT], i32),
        "pe_ap": din("pe", [S, D], f32),
        "emb_ap": din("emb", [V, D], f32),
        "embT_ap": din("embT_lm", [D, VS], bf16),
        "wqkv_ap": din("wqkvT", [n_layers, D, 384], bf16),
        "bqk_ap": din("bqk", [n_layers, 256, 1], f32),
        "vbias_ap": din("vbias", [n_layers, 128, 128], f32),
        "wo_ap": din("woT", [n_layers, D, D], bf16),
        "bo_ap": din("bo", [n_layers, D, 1], f32),
        "w1_ap": din("w1T", [n_layers, D, 1024], bf16),
        "b1_ap": din("b1", [n_layers, 1024, 1], f32),
        "w2_ap": din("w2T", [n_layers, D, 512], bf16),
        "b2_ap": din("b2", [n_layers, D, 1], f32),
        "ln1g_ap": din("ln1g", [n_layers, D, 1], f32),
        "ln1b_ap": din("ln1b", [n_layers, D, 1], f32),
        "ln2g_ap": din("ln2g", [n_layers, D, 1], f32),
        "ln2b_ap": din("ln2b", [n_layers, D, 1], f32),
        "lnfg_ap": din("lnfg", [D, 1], f32),
        "lnfb_ap": din("lnfb", [D, 1], f32),
        "mask_ap": din("mask", [NMASK, 128, 512], bf16),
        "out_ap": nc.dram_tensor("logits", [S, VS], f32, kind="ExternalOutput").ap(),
        "tap_aps": {t: nc.dram_tensor(f"tap_{t}", [D, S], f32,
                                      kind="ExternalOutput").ap() for t in taps},
        "ag_in": [nc.dram_tensor(f"ag_in{l}", [128, S], bf16, kind="Internal").ap()
                  for l in range(n_layers)],
        "ag_out": [nc.dram_tensor(f"ag_out{l}", [D, S], bf16, kind="Internal").ap()
                   for l in range(n_layers)],
        "ar_in": [nc.dram_tensor(f"ar_in{l}", [D, S], bf16, kind="Internal").ap()
                  for l in range(n_layers)],
        "ar_out": [nc.dram_tensor(f"ar_out{l}", [D, S], bf16, kind="Internal").ap()
                   for l in range(n_layers)],
    }

    with tile.TileContext(nc) as tc:
        _emit(tc, E)
    nc.compile()
    return nc


def _emit(tc, E):
    with ExitStack() as _ctx:
        _emit_body(tc, E, _ctx)


def _emit_body(tc, E, ctx):
    nc = tc.nc
    n_layers = E["n_layers"]
    taps = E["tap_aps"]

    const = ctx.enter_context(tc.tile_pool(name="const", bufs=1))
    resid = ctx.enter_context(tc.tile_pool(name="resid", bufs=1))
    wpool = ctx.enter_context(tc.tile_pool(name="wpool", bufs=2))
    act = ctx.enter_context(tc.tile_pool(name="act", bufs=2))   # small transients
    big = ctx.enter_context(tc.tile_pool(name="big", bufs=1))   # per-layer tensors
    lnp = ctx.enter_context(tc.tile_pool(name="lnp", bufs=1))
    ps = ctx.enter_context(tc.tile_pool(name="ps", bufs=2, space="PSUM"))
    ps_att = ctx.enter_context(tc.tile_pool(name="ps_att", bufs=2, space="PSUM"))
    ps_pv = ctx.enter_context(tc.tile_pool(name="ps_pv", bufs=2, space="PSUM"))
    ps_sm = ctx.enter_context(tc.tile_pool(name="ps_sm", bufs=2, space="PSUM"))

    # ---------------- constants
    ident = const.tile([128, 128], f32, name="ident")
    make_identity(nc, ident[:])
    ones_col = const.tile([128, 1], f32, name="ones_col")
    nc.any.memset(ones_col[:], 1.0)
    eps1 = const.tile([1, 1], f32, name="eps1")
    nc.any.memset(eps1[:], EPS)
    masks = [const.tile([128, 512], bf16, name=f"mask{i}") for i in range(NMASK)]
    for i in range(NMASK):
        nc.sync.dma_start(masks[i][:], E["mask_ap"][i])

    def load_dvec(ap_2d, pool, name):
        t = pool.tile([128, ND], f32, name=name, tag=name)
        nc.sync.dma_start(t[:], ap_2d.rearrange("(d p) one -> p (d one)", p=128))
        return t

    lnf_g = load_dvec(E["lnfg_ap"], const, "lnf_g")
    lnf_b = load_dvec(E["lnfb_ap"], const, "lnf_b")

    # residual stream fp32 + bf16 copy (resident)
    x = [resid.tile([128, S], f32, name=f"x{d}") for d in range(ND)]
    xb = [resid.tile([128, S], bf16, name=f"xb{d}") for d in range(ND)]

    # ---------------- embedding gather + posenc + transpose to feature-major
    idx = const.tile([128, NT], i32, name="idx")
    nc.sync.dma_start(idx[:], E["ids_ap"][:])
    for t in range(NT):
        xtm = act.tile([128, D], f32, name="xtm", tag="xtm")
        nc.gpsimd.indirect_dma_start(
            out=xtm[:], out_offset=None, in_=E["emb_ap"][:],
            in_offset=bass.IndirectOffsetOnAxis(ap=idx[:, t:t + 1], axis=0))
        petile = act.tile([128, D], f32, name="petile", tag="petile")
        nc.sync.dma_start(petile[:], E["pe_ap"][t * 128:(t + 1) * 128, :])
        nc.vector.tensor_tensor(out=xtm[:], in0=xtm[:], in1=petile[:], op=ALU.add)
        for d in range(ND):
            pt = ps_sm.tile([128, 128], f32, name="pt", tag="sm")
            nc.tensor.transpose(out=pt[:], in_=xtm[:, d * 128:(d + 1) * 128],
                                identity=ident[:])
            nc.vector.tensor_copy(out=x[d][:, t * 128:(t + 1) * 128], in_=pt[:])
            nc.scalar.copy(out=xb[d][:, t * 128:(t + 1) * 128], in_=pt[:])

    # ---------------- feature-major LayerNorm helper
    def ln_inplace(gvec, bvec, out_f32=True):
        fold = lnp.tile([128, S], f32, name="fold", tag="fold")
        nc.vector.tensor_tensor(out=fold[:], in0=x[0][:], in1=x[1][:], op=ALU.add)
        nc.vector.tensor_tensor(out=fold[:], in0=fold[:], in1=x[2][:], op=ALU.add)
        nc.vector.tensor_tensor(out=fold[:], in0=fold[:], in1=x[3][:], op=ALU.add)
        sqf = lnp.tile([128, S], f32, name="sqf", tag="sqf")
        sq = lnp.tile([128, S], f32, name="sq", tag="sq")
        nc.scalar.activation(sqf[:], x[0][:], AF.Square)
        for d in range(1, ND):
            nc.scalar.activation(sq[:], x[d][:], AF.Square)
            nc.vector.tensor_tensor(out=sqf[:], in0=sqf[:], in1=sq[:], op=ALU.add)
        mean = lnp.tile([1, S], f32, name="mean", tag="mean")
        var = lnp.tile([1, S], f32, name="var", tag="var")
        for sp in range(NSP):
            sl = slice(sp * 512, (sp + 1) * 512)
            st1 = ps_sm.tile([1, 512], f32, name="st1", tag="sm")
            nc.tensor.matmul(out=st1[:], lhsT=ones_col[:], rhs=fold[:, sl],
                             start=True, stop=True)
            nc.vector.tensor_scalar(out=mean[:, sl], in0=st1[:], scalar1=1.0 / D,
                                    scalar2=None, op0=ALU.mult)
            st2 = ps_sm.tile([1, 512], f32, name="st2", tag="sm")
            nc.tensor.matmul(out=st2[:], lhsT=ones_col[:], rhs=sqf[:, sl],
                             start=True, stop=True)
            nc.vector.tensor_scalar(out=var[:, sl], in0=st2[:], scalar1=1.0 / D,
                                    scalar2=None, op0=ALU.mult)
        msq = lnp.tile([1, S], f32, name="msq", tag="msq")
        nc.scalar.activation(msq[:], mean[:], AF.Square)
        nc.vector.tensor_tensor(out=var[:], in0=var[:], in1=msq[:], op=ALU.subtract)
        std = lnp.tile([1, S], f32, name="std", tag="std")
        nc.scalar.activation(std[:], var[:], AF.Sqrt, bias=eps1[:], scale=1.0)
        rstd = lnp.tile([1, S], f32, name="rstd", tag="rstd")
        nc.vector.reciprocal(out=rstd[:], in_=std[:])
        mrs = lnp.tile([1, S], f32, name="mrs", tag="mrs")
        nc.vector.tensor_tensor(out=mrs[:], in0=mean[:], in1=rstd[:], op=ALU.mult)
        rstd_b = lnp.tile([128, S], f32, name="rstd_b", tag="rstd_b")
        nc.gpsimd.partition_broadcast(rstd_b[:], rstd[:])
        mrs_b = lnp.tile([128, S], f32, name="mrs_b", tag="mrs_b")
        nc.gpsimd.partition_broadcast(mrs_b[:], mrs[:])
        for d in range(ND):
            t1 = lnp.tile([128, S], f32, name="t1", tag="fold")
            nc.vector.tensor_tensor(out=t1[:], in0=x[d][:], in1=rstd_b[:], op=ALU.mult)
            nc.vector.tensor_tensor(out=t1[:], in0=t1[:], in1=mrs_b[:], op=ALU.subtract)
            nc.vector.tensor_scalar(out=xb[d][:], in0=t1[:],
                                    scalar1=gvec[:, d:d + 1], scalar2=bvec[:, d:d + 1],
                                    op0=ALU.mult, op1=ALU.add)
            if out_f32:
                nc.vector.tensor_scalar(out=x[d][:], in0=t1[:],
                                        scalar1=gvec[:, d:d + 1], scalar2=bvec[:, d:d + 1],
                                        op0=ALU.mult, op1=ALU.add)

    def tap(name, from_xb=False):
        if name not in taps:
            return
        for d in range(ND):
            if from_xb:
                tf = lnp.tile([128, S], f32, name="tapf", tag="fold")
                nc.vector.tensor_copy(out=tf[:], in_=xb[d][:])
                nc.sync.dma_start(taps[name][d * 128:(d + 1) * 128, :], tf[:])
            else:
                nc.sync.dma_start(taps[name][d * 128:(d + 1) * 128, :], x[d][:])

    # ---------------- layers
    for l in range(n_layers):
        wqkv = wpool.tile([128, ND, 384], bf16, name="wqkv", tag="wqkv")
        nc.sync.dma_start(wqkv[:], E["wqkv_ap"][l].rearrange("(k p) m -> p k m", p=128))
        bqk = wpool.tile([128, 2], f32, name="bqk", tag="bqk")
        nc.sync.dma_start(bqk[:], E["bqk_ap"][l].rearrange("(a p) one -> p (a one)", p=128))
        vbias = wpool.tile([128, 128], f32, name="vbias", tag="vbias")
        nc.sync.dma_start(vbias[:], E["vbias_ap"][l])

        # q, k feature-major [128, S] bf16 (rows: slotA 0:64, slotB 64:128)
        q_sb = big.tile([128, S], bf16, name="q_sb", tag="q_sb")
        k_sb = big.tile([128, S], bf16, name="k_sb", tag="k_sb")
        for mi, dest in ((0, q_sb), (1, k_sb)):
            for sp in range(NSP):
                sl = slice(sp * 512, (sp + 1) * 512)
                pm = ps.tile([128, 512], f32, name="pm_qk", tag="mm")
                for k in range(ND):
                    nc.tensor.matmul(out=pm[:], lhsT=wqkv[:, k, mi * 128:(mi + 1) * 128],
                                     rhs=xb[k][:, sl], start=(k == 0), stop=(k == ND - 1))
                nc.scalar.activation(dest[:, sl], pm[:], AF.Identity,
                                     bias=bqk[:, mi:mi + 1])

        # v token-major per tok-tile: [128, 130] = [vA(64) | 1 | vB(64) | 1]
        vts = []
        for t in range(NT):
            pv = ps.tile([128, 128], f32, name="pv_v", tag="mm")
            for k in range(ND):
                nc.tensor.matmul(out=pv[:], lhsT=xb[k][:, t * 128:(t + 1) * 128],
                                 rhs=wqkv[:, k, 256:384],
                                 start=(k == 0), stop=(k == ND - 1))
            vsb = big.tile([128, 130], bf16, name=f"v65_{t}", tag=f"v65_{t}")
            nc.vector.tensor_tensor(out=vsb[:, 0:64], in0=pv[:, 0:64],
                                    in1=vbias[:, 0:64], op=ALU.add)
            nc.vector.tensor_tensor(out=vsb[:, 65:129], in0=pv[:, 64:128],
                                    in1=vbias[:, 64:128], op=ALU.add)
            nc.any.memset(vsb[:, 64:65], 1.0)
            nc.any.memset(vsb[:, 129:130], 1.0)
            vts.append(vsb)

        # attention: fixed 22 k-tile schedule; masks supply windows + padding
        a_sb = big.tile([128, S], bf16, name="a_sb", tag="a_sb")
        mi_idx = 0
        for slot in (0, 1):
            rows = slice(slot * 64, slot * 64 + 64)
            by_span = {}
            for sp, t in SLOT_KT[slot]:
                by_span.setdefault(sp, []).append(t)
            for sp, kts in by_span.items():
                qsl = slice(sp * 512, (sp + 1) * 512)
                pvp = ps_pv.tile([65, 512], f32, name="pvp", tag="pvp")
                for i, t in enumerate(kts):
                    scp = ps_att.tile([128, 512], f32, name="scp", tag="scp")
                    nc.tensor.matmul(out=scp[:], lhsT=k_sb[rows, t * 128:(t + 1) * 128],
                                     rhs=q_sb[rows, qsl], start=True, stop=True)
                    nc.vector.tensor_tensor(out=scp[:], in0=scp[:],
                                            in1=masks[mi_idx][:], op=ALU.add)
                    mi_idx += 1
                    p_sb = act.tile([128, 512], bf16, name="p_sb", tag="p_sb")
                    nc.scalar.activation(p_sb[:], scp[:], AF.Exp)
                    nc.tensor.matmul(out=pvp[:],
                                     lhsT=vts[t][:, slot * 65:slot * 65 + 65],
                                     rhs=p_sb[:], start=(i == 0), stop=(i == len(kts) - 1))
                den = act.tile([1, 512], f32, name="den", tag="den")
                nc.vector.reciprocal(out=den[:], in_=pvp[64:65, :])
                den_b = act.tile([64, 512], f32, name="den_b", tag="den_b")
                nc.gpsimd.partition_broadcast(den_b[:], den[:])
                nc.vector.tensor_tensor(out=a_sb[rows, qsl], in0=pvp[0:64, :],
                                        in1=den_b[:], op=ALU.mult)

        # AllGather heads within group
        nc.gpsimd.dma_start(E["ag_in"][l][:], a_sb[:])
        nc.gpsimd.collective_compute(
            "AllGather", ALU.bypass, replica_groups=E["groups"],
            ins=[E["ag_in"][l][:].opt()], outs=[E["ag_out"][l][:].opt()])
        a_full = [big.tile([128, S], bf16, name=f"a_full{d}", tag=f"a_full{d}")
                  for d in range(ND)]
        for d in range(ND):
            nc.gpsimd.dma_start(a_full[d][:], E["ag_out"][l][d * 128:(d + 1) * 128, :])

        # Wo + bias + residual into x
        wo = wpool.tile([128, ND, D], bf16, name="wo", tag="wo")
        nc.sync.dma_start(wo[:], E["wo_ap"][l].rearrange("(k p) m -> p k m", p=128))
        bo_t = load_dvec(E["bo_ap"][l], wpool, "bo_t")
        for m in range(ND):
            for sp in range(NSP):
                sl = slice(sp * 512, (sp + 1) * 512)
                pm = ps.tile([128, 512], f32, name="pm_wo", tag="mm")
                for k in range(ND):
                    nc.tensor.matmul(out=pm[:], lhsT=wo[:, k, m * 128:(m + 1) * 128],
                                     rhs=a_full[k][:, sl],
                                     start=(k == 0), stop=(k == ND - 1))
                osb = act.tile([128, 512], f32, name="osb", tag="osb")
                nc.vector.tensor_scalar(out=osb[:], in0=pm[:],
                                        scalar1=bo_t[:, m:m + 1], scalar2=None,
                                        op0=ALU.add)
                nc.vector.tensor_tensor(out=x[m][:, sl], in0=x[m][:, sl],
                                        in1=osb[:], op=ALU.add)

        ln1g = load_dvec(E["ln1g_ap"][l], wpool, "ln1g")
        ln1b = load_dvec(E["ln1b_ap"][l], wpool, "ln1b")
        ln_inplace(ln1g, ln1b)
        tap(f"ln1_{l}")

        # FFN
        w1 = wpool.tile([128, ND, 1024], bf16, name="w1", tag="w1")
        nc.sync.dma_start(w1[:], E["w1_ap"][l].rearrange("(k p) m -> p k m", p=128))
        b1 = wpool.tile([128, 8], f32, name="b1", tag="b1")
        nc.sync.dma_start(b1[:], E["b1_ap"][l].rearrange("(a p) one -> p (a one)", p=128))
        hsb = [big.tile([128, S], bf16, name=f"hsb{m}", tag=f"hsb{m}") for m in range(ND)]
        for m in range(ND):
            for sp in range(NSP):
                sl = slice(sp * 512, (sp + 1) * 512)
                pu = ps.tile([128, 512], f32, name="pu", tag="mm")
                for k in range(ND):
                    nc.tensor.matmul(out=pu[:], lhsT=w1[:, k, m * 128:(m + 1) * 128],
                                     rhs=xb[k][:, sl], start=(k == 0), stop=(k == ND - 1))
                usb = act.tile([128, 512], f32, name="usb", tag="usb")
                nc.vector.tensor_scalar(out=usb[:], in0=pu[:], scalar1=b1[:, m:m + 1],
                                        scalar2=None, op0=ALU.add)
                pg = ps.tile([128, 512], f32, name="pg", tag="mm")
                for k in range(ND):
                    nc.tensor.matmul(out=pg[:],
                                     lhsT=w1[:, k, 512 + m * 128:512 + (m + 1) * 128],
                                     rhs=xb[k][:, sl], start=(k == 0), stop=(k == ND - 1))
                gsb = act.tile([128, 512], f32, name="gsb", tag="gsb")
                nc.scalar.activation(gsb[:], pg[:], AF.Silu, bias=b1[:, 4 + m:5 + m])
                nc.vector.tensor_tensor(out=hsb[m][:, sl], in0=usb[:], in1=gsb[:],
                                        op=ALU.mult)
        w2 = wpool.tile([128, ND, 512], bf16, name="w2", tag="w2")
        nc.sync.dma_start(w2[:], E["w2_ap"][l].rearrange("(k p) m -> p k m", p=128))
        for m in range(ND):
            for sp in range(NSP):
                sl = slice(sp * 512, (sp + 1) * 512)
                pf = ps.tile([128, 512], f32, name="pf", tag="mm")
                for k in range(ND):
                    nc.tensor.matmul(out=pf[:], lhsT=w2[:, k, m * 128:(m + 1) * 128],
                                     rhs=hsb[k][:, sl], start=(k == 0), stop=(k == ND - 1))
                fsb = act.tile([128, 512], bf16, name="fsb", tag="fsb")
                nc.vector.tensor_copy(out=fsb[:], in_=pf[:])
                nc.gpsimd.dma_start(E["ar_in"][l][m * 128:(m + 1) * 128, sl], fsb[:])

        nc.gpsimd.collective_compute(
            "AllReduce", ALU.add, replica_groups=E["groups"],
            ins=[E["ar_in"][l][:].opt()], outs=[E["ar_out"][l][:].opt()])
        b2 = load_dvec(E["b2_ap"][l], wpool, "b2")
        for m in range(ND):
            ffs = act.tile([128, S], bf16, name="ffs", tag="ffs")
            nc.gpsimd.dma_start(ffs[:], E["ar_out"][l][m * 128:(m + 1) * 128, :])
            nc.vector.tensor_tensor(out=x[m][:], in0=x[m][:], in1=ffs[:], op=ALU.add)
            nc.vector.tensor_scalar(out=x[m][:], in0=x[m][:], scalar1=b2[:, m:m + 1],
                                    scalar2=None, op0=ALU.add)

        ln2g = load_dvec(E["ln2g_ap"][l], wpool, "ln2g")
        ln2b = load_dvec(E["ln2b_ap"][l], wpool, "ln2b")
        ln_inplace(ln2g, ln2b)
        tap(f"ln2_{l}")

    # final LN + lm_head (token-major output via swapped operands)
    ln_inplace(lnf_g, lnf_b, out_f32=False)
    tap("lnf", from_xb=True)
    for vs in range(NVS):
        wlm = wpool.tile([128, ND, 500], bf16, name="wlm", tag="wlm")
        nc.sync.dma_start(wlm[:], E["embT_ap"][:, vs * 500:(vs + 1) * 500]
                          .rearrange("(k p) n -> p k n", p=128))
        for t in range(NT):
            pl = ps.tile([128, 500], f32, name="pl", tag="mm")
            for k in range(ND):
                nc.tensor.matmul(out=pl[:], lhsT=xb[k][:, t * 128:(t + 1) * 128],
                                 rhs=wlm[:, k, :], start=(k == 0), stop=(k == ND - 1))
            lsb = act.tile([128, 500], f32, name="lsb", tag="lsb")
            nc.vector.tensor_copy(out=lsb[:], in_=pl[:])
            nc.sync.dma_start(E["out_ap"][t * 128:(t + 1) * 128,
                                          vs * 500:(vs + 1) * 500], lsb[:])


# ----------------------------------------------------------------- host prep
def _posenc():
    import math
    pos = np.arange(S, dtype=np.float32)[:, None]
    div = np.exp(np.arange(0, D, 2, dtype=np.float32) * (-math.log(10000.0) / D))
    pe = np.zeros((S, D), np.float32)
    pe[:, 0::2] = np.sin(pos * div)
    pe[:, 1::2] = np.cos(pos * div)
    return pe


def _masks_for(rank, not_pad):
    wins = ((rank + 1) * W, (8 - rank) * W)
    out = np.full((NMASK, 128, 512), NEG, np.float32)
    i = 0
    for slot in (0, 1):
        win = wins[slot]
        for sp, t in SLOT_KT[slot]:
            q = sp * 512 + np.arange(512)[None, :]
            k = t * 128 + np.arange(128)[:, None]
            rel = q - k
            valid = (rel >= 0) & (rel < win) & not_pad[t * 128:(t + 1) * 128, None]
            out[i] = np.where(valid, 0.0, NEG)
            i += 1
    return out.astype(ml_dtypes.bfloat16)


def _bf(a):
    return np.ascontiguousarray(a).astype(ml_dtypes.bfloat16)


def _f32c(a):
    return np.ascontiguousarray(np.asarray(a, np.float32))


def _prep_core(inputs, core, n_layers):
    g, r = divmod(core, TP)
    hA, hB = r, 7 - r
    ids = np.asarray(inputs["input_ids"][g]).astype(np.int32)
    emb = _f32c(inputs["emb"])
    Wqkv = _f32c(inputs["Wqkv"])
    bqkv = _f32c(inputs["bqkv"])
    Wo = _f32c(inputs["Wo"])
    bo = _f32c(inputs["bo"])
    W1 = _f32c(inputs["W1"])
    b1 = _f32c(inputs["b1"])
    W2 = _f32c(inputs["W2"])
    b2 = _f32c(inputs["b2"])

    def hcols(W_, base, h):
        return W_[:, :, base + h * HD:base + (h + 1) * HD]

    # wqkvT: [L, 512(din), 384] cols [qA qB kA kB vA vB]; q part pre-scaled 1/8
    WqkvT = Wqkv.transpose(0, 2, 1)  # [L, D(in), 3D(out)]
    wq = np.concatenate([hcols(WqkvT, 0, hA), hcols(WqkvT, 0, hB)], axis=2) / 8.0
    wk = np.concatenate([hcols(WqkvT, D, hA), hcols(WqkvT, D, hB)], axis=2)
    wv = np.concatenate([hcols(WqkvT, 2 * D, hA), hcols(WqkvT, 2 * D, hB)], axis=2)
    wqkvT = _bf(np.concatenate([wq, wk, wv], axis=2))

    def hseg(v, base, h):
        return v[:, base + h * HD:base + (h + 1) * HD]

    bq = np.concatenate([hseg(bqkv, 0, hA), hseg(bqkv, 0, hB)], axis=1) / 8.0
    bk = np.concatenate([hseg(bqkv, D, hA), hseg(bqkv, D, hB)], axis=1)
    bqk = np.ascontiguousarray(
        np.concatenate([bq, bk], axis=1)[:, :, None].astype(np.float32))
    bv = np.concatenate([hseg(bqkv, 2 * D, hA), hseg(bqkv, 2 * D, hB)], axis=1)
    vbias = np.ascontiguousarray(
        np.broadcast_to(bv[:, None, :], (bv.shape[0], 128, 128)).astype(np.float32))

    head_order = []
    for rr in range(TP):
        head_order += [rr, 7 - rr]
    col_perm = np.concatenate([np.arange(h * HD, (h + 1) * HD) for h in head_order])
    woT = _bf(Wo.transpose(0, 2, 1)[:, col_perm, :])  # [L, D(in, permuted), D(out)]

    uA = W1[:, r * 512:(r + 1) * 512, :]
    gA = W1[:, FF + r * 512:FF + (r + 1) * 512, :]
    w1T = _bf(np.concatenate([uA, gA], axis=1).transpose(0, 2, 1))
    b1s = np.ascontiguousarray(np.concatenate(
        [b1[:, r * 512:(r + 1) * 512], b1[:, FF + r * 512:FF + (r + 1) * 512]],
        axis=1)[:, :, None].astype(np.float32))
    w2T = _bf(W2[:, :, r * 512:(r + 1) * 512].transpose(0, 2, 1))

    not_pad = ids != 0
    return {
        "ids": np.ascontiguousarray(ids.reshape(NT, 128).T),
        "pe": _posenc(),
        "emb": emb,
        "embT_lm": _bf(emb[r * VS:(r + 1) * VS].T),
        "wqkvT": wqkvT[:n_layers],
        "bqk": bqk[:n_layers],
        "vbias": vbias[:n_layers],
        "woT": woT[:n_layers],
        "bo": np.ascontiguousarray(bo[:, :, None])[:n_layers],
        "w1T": w1T[:n_layers],
        "b1": b1s[:n_layers],
        "w2T": w2T[:n_layers],
        "b2": np.ascontiguousarray(b2[:, :, None])[:n_layers],
        "ln1g": np.ascontiguousarray(_f32c(inputs["ln1_g"])[:, :, None])[:n_layers],
        "ln1b": np.ascontiguousarray(_f32c(inputs["ln1_b"])[:, :, None])[:n_layers],
        "ln2g": np.ascontiguousarray(_f32c(inputs["ln2_g"])[:, :, None])[:n_layers],
        "ln2b": np.ascontiguousarray(_f32c(inputs["ln2_b"])[:, :, None])[:n_layers],
        "lnfg": np.ascontiguousarray(_f32c(inputs["lnf_g"])[:, None]),
        "lnfb": np.ascontiguousarray(_f32c(inputs["lnf_b"])[:, None]),
        "mask": _masks_for(r, not_pad),
    }


def kernel(**inputs):
    global LAST_RESULTS
    n_layers = int(os.environ.get("KERNEL_LAYERS", L))
    taps = tuple(t for t in os.environ.get("KERNEL_TAPS", "").split(",") if t)
    key = (n_layers, taps)
    if key not in _CACHE:
        _CACHE[key] = build_nc(n_layers, taps)
    nc = _CACHE[key]
    in_maps = [_prep_core(inputs, c, n_layers) for c in range(NC)]
    res = bass_utils.run_bass_kernel_spmd(nc, in_maps, core_ids=list(range(NC)))
    LAST_RESULTS = res
    out = np.empty((B, S, V), np.float32)
    for g in range(B):
        for r in range(TP):
            out[g][:, r * VS:(r + 1) * VS] = res.results[g * TP + r]["logits"]
    return out

